# revision 55
# baseline (speedup 1.0000x reference)
"""Multi-head self-attention (B=2, S=2048, D=1024, H=16) on 8 TRN2 NeuronCores.

Sharding: batch (2-way) x head-group (4-way) => each core computes 4 heads of
one batch item. Per core:
  - QKV projections for its 256 output dims (Q/K produced transposed, d-major)
  - flash-style attention with transposed scores S^T = K @ Q^T (so softmax
    rowsums come from a ones-matmul and A@V needs no transposes at all)
  - partial output projection (its 256 contraction dims of Wo^T)
Host gathers: out[b] = sum of the 4 partial projections (TP-style reduce).

Schedule: the ACT engine's exp stream (128 x ~1.03us) and the PE matmul
stream (~160us) are the two long poles; input DMA (~19us, globally
serialized) gates the start. Emission keeps PE gapless: single-queue
priority-ordered DMA; a c-outer K0+Q0 projection that consumes each x^T
piece as it lands (PSUM held across the contraction, copies on the
then-idle ACT engine); the V projection woven chunk-by-chunk inside the
first attention block; then 8 attention blocks (qb x pair) whose
per-chunk S->exp->AV pipeline is padded with single-matmul "filler"
items (K1/Q1 projections, per-q-block output projection) sized to the
ACT-pace slack. The softmax normalization copies PSUM to SBUF first (so
o_ps banks recycle in <1us), takes reciprocals of the free rowsum rows,
broadcasts them with two K=1 ones-matmuls, and multiplies in bf16 (4x
DVE mode). The final block runs its normalization/out-projection copies
on the idle ACT engine with 4 rotating PSUM slots.

All matmuls run in bf16 with f32 PSUM accumulation. Softmax skips the max
subtraction (scores are ~N(0,1) here, exp is safe in f32) which is
mathematically identical to softmax-with-max.
"""

import numpy as np
import ml_dtypes

B, S, D = 2, 2048, 1024
H, DH = 16, 64
NCORES = 8
HPC = 4            # heads per core
DPC = HPC * DH     # 256 dims per core
PAIRS = 2          # head pairs per core (2 heads / pair = 128 dims)
QB = 512           # q-block width
NQB = S // QB      # 4
NTC = S // 128     # 16 t-chunks
NDC = D // 128     # 8 contraction chunks for projections

_CACHE = {}


def _split_waits(nc, mybir, cap=1):
    """walrus in this container rejects >1 sync-wait per instruction
    (Too many sync wait commands). Split excess waits onto no-ops placed
    immediately before, on the same engine queue (same semantics)."""
    for fn in nc.m.functions:
        for bb in fn.blocks:
            newlist = []
            for inst in bb.instructions:
                si = inst.sync_info
                if si is not None and len(si.on_wait) > cap:
                    w = list(si.on_wait)
                    extra, keep = w[:-cap], w[-cap:]
                    for x in extra:
                        nop = mybir.InstNoOp(
                            name=f"I-ws-{nc.next_id()}", ins=[], outs=[]
                        )
                        nop.engine = inst.engine
                        nop.sync_info = mybir.SyncInfo(on_wait=[x], on_update=[])
                        newlist.append(nop)
                    inst.sync_info = mybir.SyncInfo(
                        on_wait=keep, on_update=si.on_update
                    )
                newlist.append(inst)
            bb.instructions[:] = newlist


def _build():
    from contextlib import ExitStack
    from collections import deque

    import concourse.bass as bass
    import concourse.mybir as mybir
    from concourse.tile import TileContext

    f32 = mybir.dt.float32
    bf16 = mybir.dt.bfloat16
    AF = mybir.ActivationFunctionType

    nc = bass.Bass("TRN2", target_bir_lowering=False, debug=False)

    xT = nc.dram_tensor("xT", [D, S], bf16, kind="ExternalInput").ap()
    wqT = nc.dram_tensor("wqT", [D, DPC], bf16, kind="ExternalInput").ap()
    wkT = nc.dram_tensor("wkT", [D, DPC], bf16, kind="ExternalInput").ap()
    wvT = nc.dram_tensor("wvT", [D, DPC], bf16, kind="ExternalInput").ap()
    woT = nc.dram_tensor("woT", [DPC, D], bf16, kind="ExternalInput").ap()
    out = nc.dram_tensor("out", [S, D], bf16, kind="ExternalOutput").ap()

    with TileContext(nc) as tc, ExitStack() as ctx:
        pers = ctx.enter_context(tc.tile_pool(name="pers", bufs=1))
        p_pool = ctx.enter_context(tc.tile_pool(name="p_pool", bufs=8))
        nrm = ctx.enter_context(tc.tile_pool(name="nrm", bufs=3))
        ob = ctx.enter_context(tc.tile_pool(name="ob", bufs=4))
        ps_s = ctx.enter_context(tc.tile_pool(name="ps_s", bufs=2, space="PSUM"))
        ps_o = ctx.enter_context(tc.tile_pool(name="ps_o", bufs=1, space="PSUM"))
        ps_op = ctx.enter_context(tc.tile_pool(name="ps_op", bufs=2, space="PSUM"))

        # ones rows for the recip broadcast matmuls (rows 32/64 read as K=1)
        ones = pers.tile([128, 64], bf16, name="ones", tag="ones")
        nc.gpsimd.memset(ones[:], 1.0)
        # warm the ACT Exp table during the idle prologue (real HW charges
        # ~1.3us for the first table load; the cost model doesn't)
        warm = pers.tile([128, 1], bf16, name="warm", tag="warm")
        nc.vector.memset(warm[:], 0.0)
        nc.scalar.activation(warm[0:1, :], warm[0:1, :], AF.Exp)

        # v[tt]: (128, 512) bf16 "AV-stationary" layout. Per pair m the
        # 256-col region [m*256 .. m*256+255]:
        #   cols   0-127 (head A): [V_A (64) | ones@64 | zeros]
        #   cols 128-255 (head B): [zeros | ones@32 | zeros | V_B@64-127]
        # so each AV matmul (M=128) also produces the softmax rowsum in a
        # spare PSUM row for free (A: row 64, B: row 32).
        v = []
        for tt in range(NTC):
            t = pers.tile([128, 512], bf16, name=f"v{tt}", tag=f"v{tt}")
            nc.gpsimd.memset(t[:], 0.0)
            for m in range(PAIRS):
                base = m * 256
                nc.gpsimd.memset(t[:, base + 64 : base + 65], 1.0)
                nc.gpsimd.memset(t[:, base + 128 + 32 : base + 128 + 33], 1.0)
            v.append(t)

        # ---- input DMAs, balanced across both hwdge queues (SP + ACT) in
        # 2-chunk pieces so the c-outer K/Q projection can consume x^T
        # chunks as they arrive. x^T lives in one big c-major tile so each
        # DMA moves a 2-chunk half in a single 3D-AP instruction.
        xt_all = pers.tile([128, NDC * S], bf16, name="xt_all", tag="xt_all")
        xtc = lambda c: xt_all[:, c * S : (c + 1) * S]
        wk = pers.tile([128, NDC * DPC], bf16, name="wk", tag="wk")
        wq = pers.tile([128, NDC * DPC], bf16, name="wq", tag="wq")
        wv = pers.tile([128, NDC * DPC], bf16, name="wv", tag="wv")
        wo = pers.tile([128, PAIRS * D], bf16, name="wo", tag="wo")

        def dma_w(queue, w_sb, w_dram, n):
            # partition dim must stay outermost in the SBUF-side AP
            queue.dma_start(
                out=w_sb[:].rearrange("p (c d) -> p c d", c=n),
                in_=w_dram[:].rearrange("(c p) d -> p c d", p=128),
            )

        def dma_x(queue, c0, lo, hi):
            queue.dma_start(
                out=xt_all[:].rearrange("p (c s) -> p c s", c=NDC)[
                    :, c0 : c0 + 2, lo:hi
                ],
                in_=xT[:].rearrange("(c p) s -> p c s", p=128)[
                    :, c0 : c0 + 2, lo:hi
                ],
            )

        # All input DMA transfers serialize on the global DMA engine pool,
        # so one queue in strict priority order beats two interleaved ones.
        # Small first pieces get the c-outer projection started ~3us in.
        def dma_w_half(w_sb, w_dram, c0):
            nc.sync.dma_start(
                out=w_sb[:, c0 * DPC : (c0 + 4) * DPC].rearrange(
                    "p (c d) -> p c d", c=4
                ),
                in_=w_dram[c0 * 128 : (c0 + 4) * 128, :].rearrange(
                    "(c p) d -> p c d", p=128
                ),
            )

        dma_w_half(wk, wkT, 0)
        dma_w_half(wq, wqT, 0)
        nc.sync.dma_start(out=xt_all[:, 0:1024], in_=xT[0:128, 0:1024])
        nc.sync.dma_start(out=xt_all[:, S : S + 1024], in_=xT[128:256, 0:1024])
        dma_x(nc.sync, 2, 0, 1024)
        dma_w_half(wk, wkT, 4)
        dma_w_half(wq, wqT, 4)
        dma_x(nc.sync, 4, 0, 1024)
        dma_x(nc.sync, 6, 0, 1024)
        dma_x(nc.sync, 0, 1024, 2048)
        dma_x(nc.sync, 2, 1024, 2048)
        dma_x(nc.sync, 4, 1024, 2048)
        dma_w(nc.sync, wv, wvT, NDC)
        dma_x(nc.sync, 6, 1024, 2048)
        dma_w(nc.sync, wo, woT, PAIRS)

        qt = [pers.tile([128, S], bf16, name=f"qt{m}", tag=f"qt{m}") for m in range(PAIRS)]
        kt = [pers.tile([128, S], bf16, name=f"kt{m}", tag=f"kt{m}") for m in range(PAIRS)]

        def proj_qk_mm(w_all, m, nb, c, dst, copy_engine, pool="s"):
            """One contraction-chunk matmul of a Q/K projection; the last
            chunk appends the PSUM->SBUF copy on copy_engine. pool="s" for
            the prologue (shares the scores tag while attention is not yet
            running); pool="op" for filler during attention (shares the
            outproj/bc tag so the scores pipeline keeps its 2 slots)."""
            if c == 0:
                if pool == "s":
                    proj_qk_mm.ps = ps_s.tile(
                        [128, 2 * QB], f32, name="s_ps", tag="s_ps"
                    )
                else:
                    proj_qk_mm.ps = ps_op.tile(
                        [128, QB], f32, name="op_ps", tag="op_ps"
                    )
            ps = proj_qk_mm.ps
            nc.tensor.matmul(
                ps[:, 0:QB],
                lhsT=w_all[:, c * DPC + m * 128 : c * DPC + (m + 1) * 128],
                rhs=xtc(c)[:, nb * QB : (nb + 1) * QB],
                start=(c == 0),
                stop=(c == NDC - 1),
            )
            if c == NDC - 1:
                if copy_engine == "act":
                    nc.scalar.copy(
                        out=dst[:, nb * QB : (nb + 1) * QB], in_=ps[:, 0:QB]
                    )
                else:
                    nc.vector.tensor_copy(
                        dst[:, nb * QB : (nb + 1) * QB], ps[:, 0:QB]
                    )

        def proj_v_mm(tt, c):
            # V-proj runs woven inside attention block (qb0, m0); its PSUM
            # lives on the op tag (2 slots, recycled every ~3.4us there).
            if c == 0:
                proj_v_mm.ps = ps_op.tile([128, QB], f32, name="op_ps", tag="op_ps")
            ps = proj_v_mm.ps
            nc.tensor.matmul(
                ps[:, 0:DPC],
                lhsT=xtc(c)[:, tt * 128 : (tt + 1) * 128],
                rhs=wv[:, c * DPC : (c + 1) * DPC],
                start=(c == 0),
                stop=(c == NDC - 1),
            )
            if c == NDC - 1:
                nc.vector.tensor_copy(v[tt][:, 0:64], ps[:, 0:64])
                nc.vector.tensor_copy(v[tt][:, 192:320], ps[:, 64:192])
                nc.vector.tensor_copy(v[tt][:, 448:512], ps[:, 192:256])

        # ---- prologue: ALL of K pair0 + Q pair0 emitted c-OUTER across two
        # phases (PSUM tiles for 4 q-blocks held across the contraction per
        # phase, using all 8 banks) so the PE consumes each x^T chunk the
        # moment its DMA lands. Copies ride ACT (idle until the exps).
        psK0 = ps_s.tile([128, 2 * QB], f32, name="s_ps", tag="s_ps")
        psQ0 = ps_op.tile([128, QB], f32, name="op_ps", tag="op_ps")
        psQ1 = ps_op.tile([128, QB], f32, name="op_ps", tag="op_ps")
        for c in range(NDC):  # first x^T halves: K nb0/nb1, Q nb0/nb1
            lhs_k = wk[:, c * DPC : c * DPC + 128]
            lhs_q = wq[:, c * DPC : c * DPC + 128]
            se = dict(start=(c == 0), stop=(c == NDC - 1))
            nc.tensor.matmul(psK0[:, 0:QB], lhsT=lhs_k, rhs=xtc(c)[:, 0:QB], **se)
            nc.tensor.matmul(
                psK0[:, QB : 2 * QB], lhsT=lhs_k, rhs=xtc(c)[:, QB : 2 * QB], **se
            )
            nc.tensor.matmul(psQ0[:], lhsT=lhs_q, rhs=xtc(c)[:, 0:QB], **se)
            nc.tensor.matmul(psQ1[:], lhsT=lhs_q, rhs=xtc(c)[:, QB : 2 * QB], **se)
        nc.scalar.copy(out=kt[0][:, 0:QB], in_=psK0[:, 0:QB])
        nc.scalar.copy(out=kt[0][:, QB : 2 * QB], in_=psK0[:, QB : 2 * QB])
        nc.scalar.copy(out=qt[0][:, 0:QB], in_=psQ0[:])
        nc.scalar.copy(out=qt[0][:, QB : 2 * QB], in_=psQ1[:])
        # phaseB runs nb-outer (all x^T is resident by now) on the PSUM
        # slots whose next users come latest (oA/oB/op — NOT s_ps, which
        # the attention S-pipeline grabs immediately), with DVE copies
        # trailing each projection so every slot frees ~0.7us after its
        # stop rather than all-at-once at phase end.
        for nb, mk in (
            (2, lambda: ps_o.tile([128, QB], f32, name="o_psA", tag="o_psA")),
            (3, lambda: ps_o.tile([128, QB], f32, name="o_psB", tag="o_psB")),
        ):
            ps = mk()
            for c in range(NDC):
                nc.tensor.matmul(
                    ps[:],
                    lhsT=wk[:, c * DPC : c * DPC + 128],
                    rhs=xtc(c)[:, nb * QB : (nb + 1) * QB],
                    start=(c == 0),
                    stop=(c == NDC - 1),
                )
            nc.vector.tensor_copy(kt[0][:, nb * QB : (nb + 1) * QB], ps[:])
        for nb in (2, 3):
            ps = ps_op.tile([128, QB], f32, name="op_ps", tag="op_ps")
            for c in range(NDC):
                nc.tensor.matmul(
                    ps[:],
                    lhsT=wq[:, c * DPC : c * DPC + 128],
                    rhs=xtc(c)[:, nb * QB : (nb + 1) * QB],
                    start=(c == 0),
                    stop=(c == NDC - 1),
                )
            nc.vector.tensor_copy(qt[0][:, nb * QB : (nb + 1) * QB], ps[:])
        # V chunk 0 lands before attention starts; 1-15 weave into (qb0,m0)
        for c in range(NDC):
            proj_v_mm(0, c)

        # ---- filler queues: lists of zero-arg lambdas, each emitting one
        # PE matmul (plus a trailing copy on the last contraction chunk).
        # m0-phase filler: K1 all, Q1 nb0/nb1 (48 matmuls / 48 slots)
        # m1-phase filler: Q1 nb2/nb3 + outproj(qb) as norms complete (64/64)
        fill_m0 = deque()
        for nb in range(NQB):
            for c in range(NDC):
                fill_m0.append(
                    lambda nb=nb, c=c: proj_qk_mm(wk, 1, nb, c, kt[1], "dve", "op")
                )
        for nb in (0, 1):
            for c in range(NDC):
                fill_m0.append(
                    lambda nb=nb, c=c: proj_qk_mm(wq, 1, nb, c, qt[1], "dve", "op")
                )

        fill_m1 = deque()
        for nb in (2, 3):
            for c in range(NDC):
                fill_m1.append(
                    lambda nb=nb, c=c: proj_qk_mm(wq, 1, nb, c, qt[1], "dve", "op")
                )

        ot = [[None] * PAIRS for _ in range(NQB)]

        def emit_outproj(qb, tail=False):
            """Partial output projection for q-block qb, as 16 single-matmul
            filler items; tail=True runs half the copies on ACT (idle)."""
            q0 = qb * QB
            items = []
            state = {}
            for qt_ in range(4):
                qq = qt_ * 128
                o_sb = ob.tile([128, D], bf16, name="o_sb", tag="o_sb")
                for nb in range(2):
                    for m in range(PAIRS):

                        def item(qq=qq, nb=nb, m=m, o_sb=o_sb, k=qt_ * 2 + nb):
                            if m == 0:
                                # the tail outproj has the whole PSUM to
                                # itself: 4 slots (op + freed s_ps) so the
                                # matmuls never wait on the copies.
                                if tail and k % 2 == 1:
                                    big = ps_s.tile(
                                        [128, 2 * QB], f32, name="s_ps", tag="s_ps"
                                    )
                                    state["ps"] = big[:, 0:QB]
                                else:
                                    state["ps"] = ps_op.tile(
                                        [128, QB], f32, name="op_ps", tag="op_ps"
                                    )
                            ps = state["ps"]
                            nc.tensor.matmul(
                                ps[:],
                                lhsT=ot[qb][m][:, qq : qq + 128],
                                rhs=wo[:, m * D + nb * QB : m * D + (nb + 1) * QB],
                                start=(m == 0),
                                stop=(m == PAIRS - 1),
                            )
                            if m == PAIRS - 1:
                                if tail and nb == 0:
                                    nc.scalar.copy(
                                        out=o_sb[:, nb * QB : (nb + 1) * QB], in_=ps[:]
                                    )
                                else:
                                    nc.vector.tensor_copy(
                                        o_sb[:, nb * QB : (nb + 1) * QB], ps[:]
                                    )
                                if nb == 1:
                                    nc.sync.dma_start(
                                        out=out[q0 + qq : q0 + qq + 128, :], in_=o_sb[:]
                                    )

                        items.append(item)
            return items

        # ---- attention blocks. Per (qb, m): 16 chunks of
        # S^T matmuls -> exp (ACT) -> AV matmuls, with filler woven in.
        # Block (qb0, m0) carries the V projection as its filler (one V
        # chunk per attention chunk, emitted just ahead of the AV needing
        # it); later blocks pop from the shared filler deques.
        def emit_block(qb, m, fill, v_weave=False):
            q0 = qb * QB
            o_psA = ps_o.tile([128, QB], f32, name="o_psA", tag="o_psA")
            o_psB = ps_o.tile([128, QB], f32, name="o_psB", tag="o_psB")

            def emit_s(t_):
                s_ps = ps_s.tile([128, 2 * QB], f32, name="s_ps", tag="s_ps")
                nc.tensor.matmul(
                    s_ps[:, 0:QB],
                    lhsT=kt[m][0:64, t_ * 128 : (t_ + 1) * 128],
                    rhs=qt[m][0:64, q0 : q0 + QB],
                )
                nc.tensor.matmul(
                    s_ps[:, QB : 2 * QB],
                    lhsT=kt[m][64:128, t_ * 128 : (t_ + 1) * 128],
                    rhs=qt[m][64:128, q0 : q0 + QB],
                )
                p_sb = p_pool.tile([128, 2 * QB], bf16, name="p_sb", tag="p_sb")
                nc.scalar.activation(p_sb[:], s_ps[:], AF.Exp, scale=0.125)
                return p_sb

            def emit_av(t_, p_sb):
                nc.tensor.matmul(
                    o_psA[:],
                    lhsT=v[t_][:, m * 256 : m * 256 + 128],
                    rhs=p_sb[:, 0:QB],
                    start=(t_ == 0),
                    stop=(t_ == NTC - 1),
                )
                nc.tensor.matmul(
                    o_psB[:],
                    lhsT=v[t_][:, m * 256 + 128 : m * 256 + 256],
                    rhs=p_sb[:, QB : 2 * QB],
                    start=(t_ == 0),
                    stop=(t_ == NTC - 1),
                )

            def pop_fill(n):
                for _ in range(n):
                    if fill:
                        fill.popleft()()

            prev = emit_s(0)
            for t_ in range(1, NTC):
                cur = emit_s(t_)
                if v_weave:
                    for c in range(NDC):
                        proj_v_mm(t_, c)
                else:
                    pop_fill(1)
                emit_av(t_ - 1, prev)
                prev = cur
            pop_fill(1)
            emit_av(NTC - 1, prev)

            # ---- normalization. Copy PSUM out first (frees the o_ps banks
            # for the next block's AVs in <1us), then normalize in bf16:
            # reciprocals of the rowsum rows, partition-broadcast via two
            # K=1 ones-matmuls, and 4x-mode DVE multiplies.
            tail = qb == NQB - 1 and m == PAIRS - 1
            o_preA = nrm.tile([128, QB], bf16, name="o_preA", tag="o_preA")
            o_preB = nrm.tile([128, QB], bf16, name="o_preB", tag="o_preB")
            nc.vector.tensor_copy(o_preA[:, :], o_psA[:, :])
            if tail:
                nc.scalar.copy(out=o_preB[:, :], in_=o_psB[:, :])
            else:
                nc.vector.tensor_copy(o_preB[:, :], o_psB[:, :])
            rec = nrm.tile([128, QB], bf16, name="rec", tag="rec")
            with nc.allow_low_precision("softmax recip in bf16"):
                nc.vector.reciprocal(rec[64:65, :], o_preA[64:65, :])
                nc.vector.reciprocal(rec[32:33, :], o_preB[32:33, :])
            bc_ps = ps_op.tile([128, QB], f32, name="op_ps", tag="op_ps")
            nc.tensor.matmul(bc_ps[0:64, :], lhsT=ones[64:65, 0:64], rhs=rec[64:65, :])
            nc.tensor.matmul(bc_ps[64:128, :], lhsT=ones[32:33, 0:64], rhs=rec[32:33, :])
            bc_sb = nrm.tile([128, QB], bf16, name="bc_sb", tag="bc_sb")
            if tail:
                nc.scalar.copy(out=bc_sb[:, :], in_=bc_ps[:, :])
            else:
                nc.vector.tensor_copy(bc_sb[:], bc_ps[:])
            o = nrm.tile([128, QB], bf16, name=f"ot{m}_{qb}", tag=f"ot{m}_{qb}", bufs=1)
            with nc.allow_low_precision("attn output tile in bf16"):
                nc.vector.tensor_mul(o[0:64, :], o_preA[0:64, :], bc_sb[0:64, :])
                nc.vector.tensor_mul(o[64:128, :], o_preB[64:128, :], bc_sb[64:128, :])
            ot[qb][m] = o

        emit_block(0, 0, fill_m0, v_weave=True)
        for qb in range(1, NQB):
            emit_block(qb, 0, fill_m0)
        for qb in range(NQB):
            if qb > 0:
                fill_m1.extend(emit_outproj(qb - 1))
            emit_block(qb, 1, fill_m1)
        while fill_m1:
            fill_m1.popleft()()
        for item in emit_outproj(NQB - 1, tail=True):
            item()

    _split_waits(nc, mybir)
    return nc


def _get_nc():
    if "nc" not in _CACHE:
        _CACHE["nc"] = _build()
    return _CACHE["nc"]


def _make_in_maps(x, Wq, Wk, Wv, Wo):
    bf = ml_dtypes.bfloat16
    in_maps = []
    xTb = [np.ascontiguousarray(x[b].T).astype(bf) for b in range(B)]
    for c in range(NCORES):
        b, g = divmod(c, HPC)
        lo, hi = g * DPC, (g + 1) * DPC
        in_maps.append(
            {
                "xT": xTb[b],
                "wqT": np.ascontiguousarray(Wq[lo:hi, :].T).astype(bf),
                "wkT": np.ascontiguousarray(Wk[lo:hi, :].T).astype(bf),
                "wvT": np.ascontiguousarray(Wv[lo:hi, :].T).astype(bf),
                "woT": np.ascontiguousarray(Wo[:, lo:hi].T).astype(bf),
            }
        )
    return in_maps


def _run(in_maps):
    from concourse.bass_utils import run_bass_kernel_spmd

    nc = _get_nc()
    return run_bass_kernel_spmd(nc, in_maps, core_ids=list(range(NCORES)))


def kernel(x, mask, Wq, bq, Wk, bk, Wv, bv, Wo, bo, **_ignored):
    x = np.asarray(x, dtype=np.float32)
    mask = np.asarray(mask, dtype=np.float32)
    Wq = np.asarray(Wq, dtype=np.float32)
    Wk = np.asarray(Wk, dtype=np.float32)
    Wv = np.asarray(Wv, dtype=np.float32)
    Wo = np.asarray(Wo, dtype=np.float32)
    bq = np.asarray(bq, dtype=np.float32)
    bk = np.asarray(bk, dtype=np.float32)
    bv = np.asarray(bv, dtype=np.float32)
    bo = np.asarray(bo, dtype=np.float32)

    # The fast device path assumes the trivial mask (all nonzero) and zero
    # q/k biases (true for this problem's inputs). Anything else falls back
    # to an exact host computation.
    if np.any(mask == 0) or np.any(bq) or np.any(bk):
        return _host_reference(x, mask, Wq, bq, Wk, bk, Wv, bv, Wo, bo)

    res = _run(_make_in_maps(x, Wq, Wk, Wv, Wo))

    out = np.zeros((B, S, D), dtype=np.float32)
    for c in range(NCORES):
        b = c // HPC
        out[b] += np.asarray(res.results[c]["out"], dtype=np.float32)
    # bv folds through the (row-stochastic) attention and the linear output
    # projection into a constant row; bo is a plain constant row.
    out += (bv @ Wo.T + bo).astype(np.float32)
    return out


def _host_reference(x, mask, Wq, bq, Wk, bk, Wv, bv, Wo, bo):
    Bn, Sn, Dn = x.shape
    xf = x.reshape(-1, Dn)
    Q = (xf @ Wq.T + bq).reshape(Bn, Sn, H, DH).transpose(0, 2, 1, 3)
    K = (xf @ Wk.T + bk).reshape(Bn, Sn, H, DH).transpose(0, 2, 1, 3)
    V = (xf @ Wv.T + bv).reshape(Bn, Sn, H, DH).transpose(0, 2, 1, 3)
    scores = np.einsum("bhsd,bhtd->bhst", Q, K) / np.sqrt(np.float32(DH))
    scores = np.where(mask == 0, np.float32(-1e9), scores)
    scores -= scores.max(axis=-1, keepdims=True)
    e = np.exp(scores)
    attn = e / e.sum(axis=-1, keepdims=True)
    o = np.einsum("bhst,bhtd->bhsd", attn, V)
    comb = o.transpose(0, 2, 1, 3).reshape(Bn, Sn, Dn)
    return (comb @ Wo.T + bo).astype(np.float32)


# revision 61
# speedup vs baseline: 1.0085x; 1.0085x over previous
"""Multi-head self-attention (B=2, S=2048, D=1024, H=16) on 8 TRN2 NeuronCores.

Sharding: batch (2-way) x head-group (4-way) => each core computes 4 heads of
one batch item. Per core:
  - QKV projections for its 256 output dims (Q/K produced transposed, d-major)
  - flash-style attention with transposed scores S^T = K @ Q^T (so softmax
    rowsums come from a ones-matmul and A@V needs no transposes at all)
  - partial output projection (its 256 contraction dims of Wo^T)
Host gathers: out[b] = sum of the 4 partial projections (TP-style reduce).

Schedule: the ACT engine's exp stream (128 x ~1.03us) and the PE matmul
stream (~160us) are the two long poles; input DMA (~19us, globally
serialized) gates the start. Emission keeps PE gapless: single-queue
priority-ordered DMA; a c-outer K0+Q0 projection that consumes each x^T
piece as it lands (PSUM held across the contraction, copies on the
then-idle ACT engine); the V projection woven chunk-by-chunk inside the
first attention block; then 8 attention blocks (qb x pair) whose
per-chunk S->exp->AV pipeline is padded with single-matmul "filler"
items (K1/Q1 projections, per-q-block output projection) sized to the
ACT-pace slack. The softmax normalization copies PSUM to SBUF first (so
o_ps banks recycle in <1us), takes reciprocals of the free rowsum rows,
broadcasts them with two K=1 ones-matmuls, and multiplies in bf16 (4x
DVE mode). The final block runs its normalization/out-projection copies
on the idle ACT engine with 4 rotating PSUM slots.

All matmuls run in bf16 with f32 PSUM accumulation. Softmax skips the max
subtraction (scores are ~N(0,1) here, exp is safe in f32) which is
mathematically identical to softmax-with-max.
"""

import numpy as np
import ml_dtypes

B, S, D = 2, 2048, 1024
H, DH = 16, 64
NCORES = 8
HPC = 4            # heads per core
DPC = HPC * DH     # 256 dims per core
PAIRS = 2          # head pairs per core (2 heads / pair = 128 dims)
QB = 512           # q-block width
NQB = S // QB      # 4
NTC = S // 128     # 16 t-chunks
NDC = D // 128     # 8 contraction chunks for projections

_CACHE = {}


def _split_waits(nc, mybir, cap=1):
    """walrus in this container rejects >1 sync-wait per instruction
    (Too many sync wait commands). Split excess waits onto no-ops placed
    immediately before, on the same engine queue (same semantics)."""
    for fn in nc.m.functions:
        for bb in fn.blocks:
            newlist = []
            for inst in bb.instructions:
                si = inst.sync_info
                if si is not None and len(si.on_wait) > cap:
                    w = list(si.on_wait)
                    extra, keep = w[:-cap], w[-cap:]
                    for x in extra:
                        nop = mybir.InstNoOp(
                            name=f"I-ws-{nc.next_id()}", ins=[], outs=[]
                        )
                        nop.engine = inst.engine
                        nop.sync_info = mybir.SyncInfo(on_wait=[x], on_update=[])
                        newlist.append(nop)
                    inst.sync_info = mybir.SyncInfo(
                        on_wait=keep, on_update=si.on_update
                    )
                newlist.append(inst)
            bb.instructions[:] = newlist


def _build():
    from contextlib import ExitStack
    from collections import deque

    import concourse.bass as bass
    import concourse.mybir as mybir
    from concourse.tile import TileContext

    f32 = mybir.dt.float32
    bf16 = mybir.dt.bfloat16
    AF = mybir.ActivationFunctionType

    nc = bass.Bass("TRN2", target_bir_lowering=False, debug=False)

    from concourse.bass import AP as _AP

    xT = nc.dram_tensor("xT", [D, S], bf16, kind="ExternalInput").ap()
    # DRAM scratch for the 1/rowsum partition-broadcast round-trip (2 rows
    # per non-tail attention block; rows never reused, so no WAR hazards)
    bcscr = nc.dram_tensor("bcscr", [2 * NQB * PAIRS, 512], bf16, kind="Internal").ap()
    wqT = nc.dram_tensor("wqT", [D, DPC], bf16, kind="ExternalInput").ap()
    wkT = nc.dram_tensor("wkT", [D, DPC], bf16, kind="ExternalInput").ap()
    wvT = nc.dram_tensor("wvT", [D, DPC], bf16, kind="ExternalInput").ap()
    woT = nc.dram_tensor("woT", [DPC, D], bf16, kind="ExternalInput").ap()
    out = nc.dram_tensor("out", [S, D], bf16, kind="ExternalOutput").ap()

    with TileContext(nc) as tc, ExitStack() as ctx:
        pers = ctx.enter_context(tc.tile_pool(name="pers", bufs=1))
        p_pool = ctx.enter_context(tc.tile_pool(name="p_pool", bufs=8))
        nrm = ctx.enter_context(tc.tile_pool(name="nrm", bufs=3))
        ob = ctx.enter_context(tc.tile_pool(name="ob", bufs=4))
        ps_s = ctx.enter_context(tc.tile_pool(name="ps_s", bufs=2, space="PSUM"))
        ps_o = ctx.enter_context(tc.tile_pool(name="ps_o", bufs=1, space="PSUM"))
        ps_op = ctx.enter_context(tc.tile_pool(name="ps_op", bufs=2, space="PSUM"))

        # ones rows for the recip broadcast matmuls (rows 32/64 read as K=1)
        ones = pers.tile([128, 64], bf16, name="ones", tag="ones")
        nc.gpsimd.memset(ones[:], 1.0)
        # warm the ACT Exp table during the idle prologue (real HW charges
        # ~1.3us for the first table load; the cost model doesn't)
        warm = pers.tile([128, 1], bf16, name="warm", tag="warm")
        nc.vector.memset(warm[:], 0.0)
        nc.scalar.activation(warm[0:1, :], warm[0:1, :], AF.Exp)

        # v[tt]: (128, 512) bf16 "AV-stationary" layout. Per pair m the
        # 256-col region [m*256 .. m*256+255]:
        #   cols   0-127 (head A): [V_A (64) | ones@64 | zeros]
        #   cols 128-255 (head B): [zeros | ones@32 | zeros | V_B@64-127]
        # so each AV matmul (M=128) also produces the softmax rowsum in a
        # spare PSUM row for free (A: row 64, B: row 32).
        v = []
        for tt in range(NTC):
            t = pers.tile([128, 512], bf16, name=f"v{tt}", tag=f"v{tt}")
            nc.gpsimd.memset(t[:], 0.0)
            for m in range(PAIRS):
                base = m * 256
                nc.gpsimd.memset(t[:, base + 64 : base + 65], 1.0)
                nc.gpsimd.memset(t[:, base + 128 + 32 : base + 128 + 33], 1.0)
            v.append(t)

        # ---- input DMAs, balanced across both hwdge queues (SP + ACT) in
        # 2-chunk pieces so the c-outer K/Q projection can consume x^T
        # chunks as they arrive. x^T lives in one big c-major tile so each
        # DMA moves a 2-chunk half in a single 3D-AP instruction.
        xt_all = pers.tile([128, NDC * S], bf16, name="xt_all", tag="xt_all")
        xtc = lambda c: xt_all[:, c * S : (c + 1) * S]
        wk = pers.tile([128, NDC * DPC], bf16, name="wk", tag="wk")
        wq = pers.tile([128, NDC * DPC], bf16, name="wq", tag="wq")
        wv = pers.tile([128, NDC * DPC], bf16, name="wv", tag="wv")
        wo = pers.tile([128, PAIRS * D], bf16, name="wo", tag="wo")

        def dma_w(queue, w_sb, w_dram, n):
            # partition dim must stay outermost in the SBUF-side AP
            queue.dma_start(
                out=w_sb[:].rearrange("p (c d) -> p c d", c=n),
                in_=w_dram[:].rearrange("(c p) d -> p c d", p=128),
            )

        def dma_x(queue, c0, lo, hi):
            queue.dma_start(
                out=xt_all[:].rearrange("p (c s) -> p c s", c=NDC)[
                    :, c0 : c0 + 2, lo:hi
                ],
                in_=xT[:].rearrange("(c p) s -> p c s", p=128)[
                    :, c0 : c0 + 2, lo:hi
                ],
            )

        # All input DMA transfers serialize on the global DMA engine pool,
        # so one queue in strict priority order beats two interleaved ones.
        # Small first pieces get the c-outer projection started ~3us in.
        def dma_w_half(w_sb, w_dram, c0):
            nc.sync.dma_start(
                out=w_sb[:, c0 * DPC : (c0 + 4) * DPC].rearrange(
                    "p (c d) -> p c d", c=4
                ),
                in_=w_dram[c0 * 128 : (c0 + 4) * 128, :].rearrange(
                    "(c p) d -> p c d", p=128
                ),
            )

        dma_w_half(wk, wkT, 0)
        dma_w_half(wq, wqT, 0)
        nc.sync.dma_start(out=xt_all[:, 0:1024], in_=xT[0:128, 0:1024])
        nc.sync.dma_start(out=xt_all[:, S : S + 1024], in_=xT[128:256, 0:1024])
        dma_x(nc.sync, 2, 0, 1024)
        dma_w_half(wk, wkT, 4)
        dma_w_half(wq, wqT, 4)
        dma_x(nc.sync, 4, 0, 1024)
        dma_x(nc.sync, 6, 0, 1024)
        dma_x(nc.sync, 0, 1024, 2048)
        dma_x(nc.sync, 2, 1024, 2048)
        dma_x(nc.sync, 4, 1024, 2048)
        dma_w(nc.sync, wv, wvT, NDC)
        dma_x(nc.sync, 6, 1024, 2048)
        dma_w(nc.sync, wo, woT, PAIRS)

        qt = [pers.tile([128, S], bf16, name=f"qt{m}", tag=f"qt{m}") for m in range(PAIRS)]
        kt = [pers.tile([128, S], bf16, name=f"kt{m}", tag=f"kt{m}") for m in range(PAIRS)]

        def proj_qk_mm(w_all, m, nb, c, dst, copy_engine, pool="s"):
            """One contraction-chunk matmul of a Q/K projection; the last
            chunk appends the PSUM->SBUF copy on copy_engine. pool="s" for
            the prologue (shares the scores tag while attention is not yet
            running); pool="op" for filler during attention (shares the
            outproj/bc tag so the scores pipeline keeps its 2 slots)."""
            if c == 0:
                if pool == "s":
                    proj_qk_mm.ps = ps_s.tile(
                        [128, 2 * QB], f32, name="s_ps", tag="s_ps"
                    )
                else:
                    proj_qk_mm.ps = ps_op.tile(
                        [128, QB], f32, name="op_ps", tag="op_ps"
                    )
            ps = proj_qk_mm.ps
            nc.tensor.matmul(
                ps[:, 0:QB],
                lhsT=w_all[:, c * DPC + m * 128 : c * DPC + (m + 1) * 128],
                rhs=xtc(c)[:, nb * QB : (nb + 1) * QB],
                start=(c == 0),
                stop=(c == NDC - 1),
            )
            if c == NDC - 1:
                if copy_engine == "act":
                    nc.scalar.copy(
                        out=dst[:, nb * QB : (nb + 1) * QB], in_=ps[:, 0:QB]
                    )
                else:
                    nc.vector.tensor_copy(
                        dst[:, nb * QB : (nb + 1) * QB], ps[:, 0:QB]
                    )

        def proj_v_mm(tt, c):
            # V-proj runs woven inside attention block (qb0, m0); its PSUM
            # lives on the op tag (2 slots, recycled every ~3.4us there).
            if c == 0:
                proj_v_mm.ps = ps_op.tile([128, QB], f32, name="op_ps", tag="op_ps")
            ps = proj_v_mm.ps
            nc.tensor.matmul(
                ps[:, 0:DPC],
                lhsT=xtc(c)[:, tt * 128 : (tt + 1) * 128],
                rhs=wv[:, c * DPC : (c + 1) * DPC],
                start=(c == 0),
                stop=(c == NDC - 1),
            )
            if c == NDC - 1:
                nc.vector.tensor_copy(v[tt][:, 0:64], ps[:, 0:64])
                nc.vector.tensor_copy(v[tt][:, 192:320], ps[:, 64:192])
                nc.vector.tensor_copy(v[tt][:, 448:512], ps[:, 192:256])

        # ---- prologue: ALL of K pair0 + Q pair0 emitted c-OUTER across two
        # phases (PSUM tiles for 4 q-blocks held across the contraction per
        # phase, using all 8 banks) so the PE consumes each x^T chunk the
        # moment its DMA lands. Copies ride ACT (idle until the exps).
        psK0 = ps_s.tile([128, 2 * QB], f32, name="s_ps", tag="s_ps")
        psQ0 = ps_op.tile([128, QB], f32, name="op_ps", tag="op_ps")
        psQ1 = ps_op.tile([128, QB], f32, name="op_ps", tag="op_ps")
        for c in range(NDC):  # first x^T halves: K nb0/nb1, Q nb0/nb1
            lhs_k = wk[:, c * DPC : c * DPC + 128]
            lhs_q = wq[:, c * DPC : c * DPC + 128]
            se = dict(start=(c == 0), stop=(c == NDC - 1))
            nc.tensor.matmul(psK0[:, 0:QB], lhsT=lhs_k, rhs=xtc(c)[:, 0:QB], **se)
            nc.tensor.matmul(
                psK0[:, QB : 2 * QB], lhsT=lhs_k, rhs=xtc(c)[:, QB : 2 * QB], **se
            )
            nc.tensor.matmul(psQ0[:], lhsT=lhs_q, rhs=xtc(c)[:, 0:QB], **se)
            nc.tensor.matmul(psQ1[:], lhsT=lhs_q, rhs=xtc(c)[:, QB : 2 * QB], **se)
        nc.scalar.copy(out=kt[0][:, 0:QB], in_=psK0[:, 0:QB])
        nc.scalar.copy(out=kt[0][:, QB : 2 * QB], in_=psK0[:, QB : 2 * QB])
        nc.scalar.copy(out=qt[0][:, 0:QB], in_=psQ0[:])
        nc.scalar.copy(out=qt[0][:, QB : 2 * QB], in_=psQ1[:])
        # phaseB runs nb-outer (all x^T is resident by now) on the PSUM
        # slots whose next users come latest (oA/oB/op — NOT s_ps, which
        # the attention S-pipeline grabs immediately), with DVE copies
        # trailing each projection so every slot frees ~0.7us after its
        # stop rather than all-at-once at phase end.
        for nb, mk in (
            (2, lambda: ps_o.tile([128, QB], f32, name="o_psA", tag="o_psA")),
            (3, lambda: ps_o.tile([128, QB], f32, name="o_psB", tag="o_psB")),
        ):
            ps = mk()
            for c in range(NDC):
                nc.tensor.matmul(
                    ps[:],
                    lhsT=wk[:, c * DPC : c * DPC + 128],
                    rhs=xtc(c)[:, nb * QB : (nb + 1) * QB],
                    start=(c == 0),
                    stop=(c == NDC - 1),
                )
            nc.vector.tensor_copy(kt[0][:, nb * QB : (nb + 1) * QB], ps[:])
        for nb in (2, 3):
            ps = ps_op.tile([128, QB], f32, name="op_ps", tag="op_ps")
            for c in range(NDC):
                nc.tensor.matmul(
                    ps[:],
                    lhsT=wq[:, c * DPC : c * DPC + 128],
                    rhs=xtc(c)[:, nb * QB : (nb + 1) * QB],
                    start=(c == 0),
                    stop=(c == NDC - 1),
                )
            nc.vector.tensor_copy(qt[0][:, nb * QB : (nb + 1) * QB], ps[:])
        # V chunk 0 lands before attention starts; 1-15 weave into (qb0,m0)
        for c in range(NDC):
            proj_v_mm(0, c)

        # ---- filler queues: lists of zero-arg lambdas, each emitting one
        # PE matmul (plus a trailing copy on the last contraction chunk).
        # m0-phase filler: K1 all, Q1 nb0/nb1 (48 matmuls / 48 slots)
        # m1-phase filler: Q1 nb2/nb3 + outproj(qb) as norms complete (64/64)
        fill_m0 = deque()
        for nb in range(NQB):
            for c in range(NDC):
                fill_m0.append(
                    lambda nb=nb, c=c: proj_qk_mm(wk, 1, nb, c, kt[1], "dve", "op")
                )
        for nb in (0, 1):
            for c in range(NDC):
                fill_m0.append(
                    lambda nb=nb, c=c: proj_qk_mm(wq, 1, nb, c, qt[1], "dve", "op")
                )

        fill_m1 = deque()
        for nb in (2, 3):
            for c in range(NDC):
                fill_m1.append(
                    lambda nb=nb, c=c: proj_qk_mm(wq, 1, nb, c, qt[1], "dve", "op")
                )

        ot = [[None] * PAIRS for _ in range(NQB)]

        def emit_outproj(qb, tail=False):
            """Partial output projection for q-block qb, as 16 single-matmul
            filler items; tail=True runs half the copies on ACT (idle)."""
            q0 = qb * QB
            items = []
            state = {}
            for qt_ in range(4):
                qq = qt_ * 128
                o_sb = ob.tile([128, D], bf16, name="o_sb", tag="o_sb")
                for nb in range(2):
                    for m in range(PAIRS):

                        def item(qq=qq, nb=nb, m=m, o_sb=o_sb, k=qt_ * 2 + nb):
                            if m == 0:
                                # the tail outproj has the whole PSUM to
                                # itself: 4 slots (op + freed s_ps) so the
                                # matmuls never wait on the copies.
                                if tail and k % 2 == 1:
                                    big = ps_s.tile(
                                        [128, 2 * QB], f32, name="s_ps", tag="s_ps"
                                    )
                                    state["ps"] = big[:, 0:QB]
                                else:
                                    state["ps"] = ps_op.tile(
                                        [128, QB], f32, name="op_ps", tag="op_ps"
                                    )
                            ps = state["ps"]
                            nc.tensor.matmul(
                                ps[:],
                                lhsT=ot[qb][m][:, qq : qq + 128],
                                rhs=wo[:, m * D + nb * QB : m * D + (nb + 1) * QB],
                                start=(m == 0),
                                stop=(m == PAIRS - 1),
                            )
                            if m == PAIRS - 1:
                                if tail and nb == 0:
                                    nc.scalar.copy(
                                        out=o_sb[:, nb * QB : (nb + 1) * QB], in_=ps[:]
                                    )
                                else:
                                    nc.vector.tensor_copy(
                                        o_sb[:, nb * QB : (nb + 1) * QB], ps[:]
                                    )
                                if nb == 1:
                                    nc.sync.dma_start(
                                        out=out[q0 + qq : q0 + qq + 128, :], in_=o_sb[:]
                                    )

                        items.append(item)
            return items

        # ---- attention blocks. Per (qb, m): 16 chunks of
        # S^T matmuls -> exp (ACT) -> AV matmuls, with filler woven in.
        # Block (qb0, m0) carries the V projection as its filler (one V
        # chunk per attention chunk, emitted just ahead of the AV needing
        # it); later blocks pop from the shared filler deques.
        def emit_block(qb, m, fill, v_weave=False):
            q0 = qb * QB
            o_psA = ps_o.tile([128, QB], f32, name="o_psA", tag="o_psA")
            o_psB = ps_o.tile([128, QB], f32, name="o_psB", tag="o_psB")

            def emit_s(t_):
                s_ps = ps_s.tile([128, 2 * QB], f32, name="s_ps", tag="s_ps")
                nc.tensor.matmul(
                    s_ps[:, 0:QB],
                    lhsT=kt[m][0:64, t_ * 128 : (t_ + 1) * 128],
                    rhs=qt[m][0:64, q0 : q0 + QB],
                )
                nc.tensor.matmul(
                    s_ps[:, QB : 2 * QB],
                    lhsT=kt[m][64:128, t_ * 128 : (t_ + 1) * 128],
                    rhs=qt[m][64:128, q0 : q0 + QB],
                )
                p_sb = p_pool.tile([128, 2 * QB], bf16, name="p_sb", tag="p_sb")
                nc.scalar.activation(p_sb[:], s_ps[:], AF.Exp, scale=0.125)
                return p_sb

            def emit_av(t_, p_sb):
                nc.tensor.matmul(
                    o_psA[:],
                    lhsT=v[t_][:, m * 256 : m * 256 + 128],
                    rhs=p_sb[:, 0:QB],
                    start=(t_ == 0),
                    stop=(t_ == NTC - 1),
                )
                nc.tensor.matmul(
                    o_psB[:],
                    lhsT=v[t_][:, m * 256 + 128 : m * 256 + 256],
                    rhs=p_sb[:, QB : 2 * QB],
                    start=(t_ == 0),
                    stop=(t_ == NTC - 1),
                )

            def pop_fill(n):
                for _ in range(n):
                    if fill:
                        fill.popleft()()

            prev = emit_s(0)
            for t_ in range(1, NTC):
                cur = emit_s(t_)
                if v_weave:
                    for c in range(NDC):
                        proj_v_mm(t_, c)
                else:
                    pop_fill(1)
                emit_av(t_ - 1, prev)
                prev = cur
            pop_fill(1)
            emit_av(NTC - 1, prev)

            # ---- normalization. Copy PSUM out first (frees the o_ps banks
            # for the next block's AVs in <1us), then normalize in bf16:
            # reciprocals of the rowsum rows, partition-broadcast via two
            # K=1 ones-matmuls, and 4x-mode DVE multiplies.
            tail = qb == NQB - 1 and m == PAIRS - 1
            o_preA = nrm.tile([128, QB], bf16, name="o_preA", tag="o_preA")
            o_preB = nrm.tile([128, QB], bf16, name="o_preB", tag="o_preB")
            nc.vector.tensor_copy(o_preA[:, :], o_psA[:, :])
            if tail:
                nc.scalar.copy(out=o_preB[:, :], in_=o_psB[:, :])
            else:
                nc.vector.tensor_copy(o_preB[:, :], o_psB[:, :])
            rec = nrm.tile([128, QB], bf16, name="rec", tag="rec")
            with nc.allow_low_precision("softmax recip in bf16"):
                nc.vector.reciprocal(rec[64:65, :], o_preA[64:65, :])
                nc.vector.reciprocal(rec[32:33, :], o_preB[32:33, :])
            bc_sb = nrm.tile([128, QB], bf16, name="bc_sb", tag="bc_sb")
            if tail:
                # latency-critical at the tail: broadcast via two K=1 PE
                # matmuls (PE is idle here) + one ACT copy
                bc_ps = ps_op.tile([128, QB], f32, name="op_ps", tag="op_ps")
                nc.tensor.matmul(
                    bc_ps[0:64, :], lhsT=ones[64:65, 0:64], rhs=rec[64:65, :]
                )
                nc.tensor.matmul(
                    bc_ps[64:128, :], lhsT=ones[32:33, 0:64], rhs=rec[32:33, :]
                )
                nc.scalar.copy(out=bc_sb[:, :], in_=bc_ps[:, :])
            else:
                # off the critical path: broadcast on the idle DMA engines
                # via a DRAM round-trip (rows -> scratch, then a stride-0
                # read replicating each row across 64 partitions). Same
                # queue => FIFO transfer order guarantees RAW on bcscr.
                bi = 2 * (qb + NQB * m)
                nc.sync.dma_start(out=bcscr[bi : bi + 1, :], in_=rec[64:65, :])
                nc.sync.dma_start(out=bcscr[bi + 1 : bi + 2, :], in_=rec[32:33, :])
                rowA = bcscr[bi : bi + 1, :]
                rowB = bcscr[bi + 1 : bi + 2, :]
                nc.sync.dma_start(
                    out=bc_sb[0:64, :],
                    in_=_AP(rowA.tensor, rowA.offset, [[0, 64], [1, QB]]),
                )
                nc.sync.dma_start(
                    out=bc_sb[64:128, :],
                    in_=_AP(rowB.tensor, rowB.offset, [[0, 64], [1, QB]]),
                )
            o = nrm.tile([128, QB], bf16, name=f"ot{m}_{qb}", tag=f"ot{m}_{qb}", bufs=1)
            with nc.allow_low_precision("attn output tile in bf16"):
                nc.vector.tensor_mul(o[0:64, :], o_preA[0:64, :], bc_sb[0:64, :])
                nc.vector.tensor_mul(o[64:128, :], o_preB[64:128, :], bc_sb[64:128, :])
            ot[qb][m] = o

        emit_block(0, 0, fill_m0, v_weave=True)
        for qb in range(1, NQB):
            emit_block(qb, 0, fill_m0)
        for qb in range(NQB):
            if qb > 0:
                fill_m1.extend(emit_outproj(qb - 1))
            emit_block(qb, 1, fill_m1)
        while fill_m1:
            fill_m1.popleft()()
        for item in emit_outproj(NQB - 1, tail=True):
            item()

    _split_waits(nc, mybir)
    return nc


def _get_nc():
    if "nc" not in _CACHE:
        _CACHE["nc"] = _build()
    return _CACHE["nc"]


def _make_in_maps(x, Wq, Wk, Wv, Wo):
    bf = ml_dtypes.bfloat16
    in_maps = []
    xTb = [np.ascontiguousarray(x[b].T).astype(bf) for b in range(B)]
    for c in range(NCORES):
        b, g = divmod(c, HPC)
        lo, hi = g * DPC, (g + 1) * DPC
        in_maps.append(
            {
                "xT": xTb[b],
                "wqT": np.ascontiguousarray(Wq[lo:hi, :].T).astype(bf),
                "wkT": np.ascontiguousarray(Wk[lo:hi, :].T).astype(bf),
                "wvT": np.ascontiguousarray(Wv[lo:hi, :].T).astype(bf),
                "woT": np.ascontiguousarray(Wo[:, lo:hi].T).astype(bf),
            }
        )
    return in_maps


def _run(in_maps):
    from concourse.bass_utils import run_bass_kernel_spmd

    nc = _get_nc()
    return run_bass_kernel_spmd(nc, in_maps, core_ids=list(range(NCORES)))


def kernel(x, mask, Wq, bq, Wk, bk, Wv, bv, Wo, bo, **_ignored):
    x = np.asarray(x, dtype=np.float32)
    mask = np.asarray(mask, dtype=np.float32)
    Wq = np.asarray(Wq, dtype=np.float32)
    Wk = np.asarray(Wk, dtype=np.float32)
    Wv = np.asarray(Wv, dtype=np.float32)
    Wo = np.asarray(Wo, dtype=np.float32)
    bq = np.asarray(bq, dtype=np.float32)
    bk = np.asarray(bk, dtype=np.float32)
    bv = np.asarray(bv, dtype=np.float32)
    bo = np.asarray(bo, dtype=np.float32)

    # The fast device path assumes the trivial mask (all nonzero) and zero
    # q/k biases (true for this problem's inputs). Anything else falls back
    # to an exact host computation.
    if np.any(mask == 0) or np.any(bq) or np.any(bk):
        return _host_reference(x, mask, Wq, bq, Wk, bk, Wv, bv, Wo, bo)

    res = _run(_make_in_maps(x, Wq, Wk, Wv, Wo))

    out = np.zeros((B, S, D), dtype=np.float32)
    for c in range(NCORES):
        b = c // HPC
        out[b] += np.asarray(res.results[c]["out"], dtype=np.float32)
    # bv folds through the (row-stochastic) attention and the linear output
    # projection into a constant row; bo is a plain constant row.
    out += (bv @ Wo.T + bo).astype(np.float32)
    return out


def _host_reference(x, mask, Wq, bq, Wk, bk, Wv, bv, Wo, bo):
    Bn, Sn, Dn = x.shape
    xf = x.reshape(-1, Dn)
    Q = (xf @ Wq.T + bq).reshape(Bn, Sn, H, DH).transpose(0, 2, 1, 3)
    K = (xf @ Wk.T + bk).reshape(Bn, Sn, H, DH).transpose(0, 2, 1, 3)
    V = (xf @ Wv.T + bv).reshape(Bn, Sn, H, DH).transpose(0, 2, 1, 3)
    scores = np.einsum("bhsd,bhtd->bhst", Q, K) / np.sqrt(np.float32(DH))
    scores = np.where(mask == 0, np.float32(-1e9), scores)
    scores -= scores.max(axis=-1, keepdims=True)
    e = np.exp(scores)
    attn = e / e.sum(axis=-1, keepdims=True)
    o = np.einsum("bhst,bhtd->bhsd", attn, V)
    comb = o.transpose(0, 2, 1, 3).reshape(Bn, Sn, Dn)
    return (comb @ Wo.T + bo).astype(np.float32)
